# revision 18
# baseline (speedup 1.0000x reference)
"""Trainium2 Bass kernel for nn_CAM (DANet channel-attention module).

Per batch element b (one per NeuronCore, 8 cores data-parallel over B=8):
    xf = x[b].reshape(C, H*W)                       # [512, 4096]
    E = xf @ xf.T                                   # [512, 512] (symmetric)
    att = softmax(max_j(E) - E, axis=-1)            # inverted softmax
    out = gamma * (att @ xf) + x[b]

Kernel math (identical in exact arithmetic to the reference):
    c[i]    = min_j E[i, j]                          (row min)
    N[i, j] = exp(c[i] - E[i, j])                    (numerator, exponent <= 0)
    S[i]    = sum_j N[i, j]
    out[i]  = (gamma / S[i]) * sum_j N[i, j] * xf[j, :] + x[b][i, :]

Layout strategy:
  - xf natural  [c_part, n_free]  : [128, 4, 4096] f32; the BIR verifier
                                    requires fp32r matmul operands to be
                                    PRODUCED rounded, so the otherwise-idle
                                    GpSimd engine makes the f32r copy Xr
                                    (hidden behind the input DMA)
  - xf^T        [n_part, c_free]  : per-2k-tile [128, 2, 512] f32r staging via
                                    PE transposes; software-pipelined with mm1
                                    (2-group lookahead so PE never waits on
                                    the PSUM->SBUF copies)
  - E           [j_part, i_free]  : 4 PSUM banks, fp32r matmuls over 32 k-tiles
                                    (upper block-triangle by symmetry, lower
                                    reconstructed with PE transposes)
  - N = exp(c-E): ONE ACT instruction per row-block: activation(Exp,
                  scale=-1, bias=rowmin) with accum_out producing S — no
                  free-axis broadcast of c, no DRAM roundtrip, no ones-matmuls
  - W = N^T     [j_part, i_free]  : 16 PE transposes; lhsT of mm2
  - mm2: jb-outer over 2-chunk halves (weight reuse, PSUM ping-pong)

reps > 1 unrolls the whole computation serially inside one NEFF (used by
test.py to measure steady-state per-iteration device time).
"""

import numpy as np

import concourse.bass as bass
import concourse.mybir as mybir
import concourse.tile as tile
from concourse import bacc
from concourse.masks import make_identity

P = 128          # partitions
C = 512          # channels
HW = 4096        # spatial (64*64)
CB = C // P      # 4 channel blocks
KB = HW // P     # 32 spatial blocks
NW = 512         # matmul free-dim chunk
NCH = HW // NW   # 8 n-chunks
NG = KB // 2     # 16 transpose/matmul groups (2 k-tiles each)

F32 = mybir.dt.float32
F32R = mybir.dt.float32r
# timing experiment knob: per-chunk-inner mm2 loop (2x the lhsT switches)
MM2_CHN_INNER = False
EXP = mybir.ActivationFunctionType.Exp
ALU = mybir.AluOpType
AX = mybir.AxisListType


def build_nc(reps: int = 1):
    nc = bacc.Bacc("TRN2", target_bir_lowering=False)
    x = nc.dram_tensor("x", [C, HW], F32, kind="ExternalInput")
    g = nc.dram_tensor("gamma", [1], F32, kind="ExternalInput")
    y = nc.dram_tensor("y", [C, HW], F32, kind="ExternalOutput")

    with tile.TileContext(nc) as tc:
        with (
            tc.tile_pool(name="xin", bufs=1) as xin_pool,
            tc.tile_pool(name="xtr", bufs=4) as xtr_pool,
            tc.tile_pool(name="nw", bufs=1) as nw_pool,
            tc.tile_pool(name="small", bufs=1) as small,
            tc.tile_pool(name="outp", bufs=2) as outp,
            tc.tile_pool(name="dram", bufs=1, space="DRAM") as dramp,
            tc.tile_pool(name="pxt", bufs=2, space="PSUM") as pxt_pool,
            tc.tile_pool(name="acc", bufs=4, space="PSUM") as acc_pool,
        ):
            # constants (hoisted out of the rep loop)
            ident_f = small.tile([P, P], F32)
            make_identity(nc, ident_f)
            ident = small.tile([P, P], F32R)
            nc.scalar.copy(out=ident, in_=ident_f)
            gamma_bc = small.tile([P, 1], F32)
            nc.gpsimd.dma_start(out=gamma_bc, in_=g[:].partition_broadcast(P))
            # prewarm the Exp activation table during phase 1 of rep 0
            dummy = small.tile([P, 1], F32)
            nc.scalar.activation(out=dummy, in_=gamma_bc, func=EXP)

            xr = x.rearrange("(t p) n -> p t n", p=P)
            yr = y.rearrange("(t p) n -> p t n", p=P)

            # For reps > 1 (timing variants) chain each rep's input from the
            # previous rep's output via a tracked DRAM scratch tile so the
            # compiler cannot dead-code-eliminate intermediate reps.
            if reps > 1:
                ybuf = dramp.tile([C, HW], F32, tag="ybuf")
                ybr = ybuf.rearrange("(t p) n -> p t n", p=P)

            for _rep in range(reps):
                in_r = xr if _rep == 0 else ybr
                out_r = yr if _rep == reps - 1 else ybr

                X = xin_pool.tile([P, CB, HW], F32, tag="x")
                Xr = xin_pool.tile([P, CB, HW], F32R, tag="xr")
                N = nw_pool.tile([P, CB, C], F32R, tag="n")
                W = nw_pool.tile([P, CB, C], F32R, tag="w")
                rowmin = small.tile([P, CB], F32, tag="rowmin")
                S = small.tile([P, CB], F32, tag="s")
                invsg = small.tile([P, CB], F32, tag="invsg")
                blk = small.tile([P, 5, P], F32, tag="blk")

                # E accumulator banks (held across the fused load/T/mm1 loop)
                pe_tiles = [acc_pool.tile([P, C], F32, tag="acc", name=f"pe_{_jb}")
                            for _jb in range(CB)]
                # by symmetry only the upper block-triangle of E is computed
                # by matmuls; rhs column start per j-block (block (3,2) is
                # recomputed directly so every matmul keeps free dim >= 256)
                RS = (0, P, 2 * P, 2 * P)

                # ---- all input DMAs up front; the queue streams them.
                # fp32r matmul operands must be PRODUCED rounded-to-fp32r
                # (BIR verifier rule), so the otherwise-idle GpSimd engine
                # makes the rounding copy X -> Xr chunk by chunk.
                # first chunk split in two so group-0 transposes start sooner
                nc.sync.dma_start(out=X[:, :, 0:NW // 2], in_=in_r[:, :, 0:NW // 2])
                nc.sync.dma_start(out=X[:, :, NW // 2:NW], in_=in_r[:, :, NW // 2:NW])
                for ch in range(1, NCH):
                    nsl = slice(ch * NW, (ch + 1) * NW)
                    nc.sync.dma_start(out=X[:, :, nsl], in_=in_r[:, :, nsl])
                for ch in range(NCH):
                    nsl = slice(ch * NW, (ch + 1) * NW)
                    nc.gpsimd.tensor_copy(out=Xr[:, :, nsl], in_=X[:, :, nsl])

                # ---- software-pipelined: T(g) | copy(g) | mm(g-1)
                xt_tiles = {}

                def emit_t(gi):
                    # chunk-0 groups transpose X (fp32) directly: no wait on
                    # the Xr rounding copy at the start of the rep
                    f32_path = gi < 2
                    pxt = pxt_pool.tile([P, 2, C], F32 if f32_path else F32R,
                                        tag="pxt")
                    src, idn = (X, ident_f) if f32_path else (Xr, ident)
                    for dk in range(2):
                        k = 2 * gi + dk
                        for t in range(CB):
                            nc.tensor.transpose(
                                pxt[:, dk, t * P:(t + 1) * P],
                                src[:, t, k * P:(k + 1) * P],
                                idn,
                            )
                    xt2 = xtr_pool.tile([P, 2, C], F32R, tag="xtk")
                    xt_tiles[gi] = (pxt, xt2)

                def emit_copy(gi):
                    pxt, xt2 = xt_tiles[gi]
                    src = pxt if gi < 2 else pxt.bitcast(F32)
                    if gi % 2 == 0:
                        nc.vector.tensor_copy(out=xt2, in_=src)
                    else:
                        nc.scalar.copy(out=xt2, in_=src)

                def emit_mm(gi):
                    _, xt2 = xt_tiles.pop(gi)
                    for dk in range(2):
                        k = 2 * gi + dk
                        for jb in range(CB):
                            nc.tensor.matmul(
                                pe_tiles[jb][:, RS[jb]:],
                                lhsT=xt2[:, dk, jb * P:(jb + 1) * P],
                                rhs=xt2[:, dk, RS[jb]:],
                                start=(k == 0),
                                stop=(k == KB - 1),
                            )

                for gi in range(NG):
                    emit_t(gi)
                    emit_copy(gi)
                    if gi >= 2:
                        emit_mm(gi - 2)
                emit_mm(NG - 2)
                emit_mm(NG - 1)

                # ---- boundary: row minima, lower-triangle fills, exp, 1/S
                nc.vector.tensor_reduce(
                    out=rowmin[:, 0:1], in_=pe_tiles[0], axis=AX.X, op=ALU.min
                )
                # upper-tri blocks (1,0),(2,0),(3,0) sit contiguously in
                # pe[0][:,128:512]; (2,1),(3,1) in pe[1][:,256:512]
                nc.scalar.copy(out=blk[:, 0:3, :], in_=pe_tiles[0][:, P:4 * P])
                nc.scalar.copy(out=blk[:, 3:5, :], in_=pe_tiles[1][:, 2 * P:4 * P])
                for n5, (bi, bj) in enumerate(((1, 0), (2, 0), (3, 0), (2, 1), (3, 1))):
                    nc.tensor.transpose(
                        pe_tiles[bi][:, bj * P:(bj + 1) * P], blk[:, n5, :], ident_f
                    )
                for b in (1, 2, 3):
                    nc.vector.tensor_reduce(
                        out=rowmin[:, b:b + 1], in_=pe_tiles[b], axis=AX.X, op=ALU.min
                    )
                # N[i,:] = exp(c_i - E[i,:]) and S[i] = sum of the row, fused
                for b in range(CB):
                    nc.scalar.activation(
                        out=N[:, b, :], in_=pe_tiles[b],
                        func=EXP, scale=-1.0, bias=rowmin[:, b:b + 1],
                        accum_out=S[:, b:b + 1],
                    )
                nc.vector.reciprocal(out=invsg, in_=S)
                nc.vector.tensor_scalar(
                    out=invsg, in0=invsg, scalar1=gamma_bc[:, 0:1], scalar2=None,
                    op0=ALU.mult,
                )

                # ---- W = N^T (lhsT layout for mm2)
                for jb in range(CB):
                    pw = acc_pool.tile([P, C], F32R, tag="acc",
                                       name=f"pw_{jb}")
                    for ib in range(CB):
                        nc.tensor.transpose(
                            pw[:, ib * P:(ib + 1) * P],
                            N[:, ib, jb * P:(jb + 1) * P],
                            ident,
                        )
                    if jb % 2 == 0:
                        nc.vector.tensor_copy(out=W[:, jb, :], in_=pw.bitcast(F32))
                    else:
                        nc.scalar.copy(out=W[:, jb, :], in_=pw.bitcast(F32))

                # ---- phase 2: out = (gamma/S) * (N^T)^T... = att-weighted sum
                # jb-outer over 2-chunk halves: each lhsT loaded once per half,
                # PSUM ping-pongs (2 live + 2 draining within acc bufs=4)
                for ib in range(CB):
                    isl = slice(ib * P, (ib + 1) * P)
                    out_sb = outp.tile([P, HW], F32, tag="osb")
                    for half in range(NCH // 2):
                        hsl = slice(2 * half * NW, (2 * half + 2) * NW)
                        # 2-bank PSUM pair from the (phase-2 idle) pxt pool
                        po = pxt_pool.tile([P, 2, NW], F32, tag="pxt",
                                           name=f"po_{ib}_{half}")
                        if MM2_CHN_INNER:
                            mm_iter = [(jb, pi) for pi in range(2)
                                       for jb in range(CB)]
                        else:
                            mm_iter = [(jb, pi) for jb in range(CB)
                                       for pi in range(2)]
                        for jb, pi in mm_iter:
                            chn = 2 * half + pi
                            nsl = slice(chn * NW, (chn + 1) * NW)
                            nc.tensor.matmul(
                                po[:, pi, :],
                                lhsT=W[:, jb, isl],
                                rhs=Xr[:, jb, nsl],
                                start=(jb == 0),
                                stop=(jb == CB - 1),
                            )
                        nc.vector.scalar_tensor_tensor(
                            out=out_sb[:, hsl],
                            in0=po,
                            scalar=invsg[:, ib:ib + 1],
                            in1=X[:, ib, hsl],
                            op0=ALU.mult,
                            op1=ALU.add,
                        )
                        if half % 2 == 1:
                            osl = slice((2 * half - 2) * NW, (2 * half + 2) * NW)
                            nc.scalar.dma_start(
                                out=out_r[:, ib, osl], in_=out_sb[:, osl]
                            )

    nc.compile()
    return nc


_NC_CACHE = None


def _get_nc():
    global _NC_CACHE
    if _NC_CACHE is None:
        _NC_CACHE = build_nc()
    return _NC_CACHE


def kernel(x, gamma):
    from concourse.bass_utils import run_bass_kernel_spmd

    x = np.ascontiguousarray(np.asarray(x, dtype=np.float32))
    B = x.shape[0]
    assert x.shape == (8, C, 64, 64), x.shape
    xf = x.reshape(B, C, HW)
    gamma = np.ascontiguousarray(np.asarray(gamma, dtype=np.float32)).reshape(1)

    nc = _get_nc()
    in_maps = [{"x": xf[b], "gamma": gamma} for b in range(B)]
    res = run_bass_kernel_spmd(nc, in_maps, core_ids=list(range(B)))
    out = np.stack([res.results[b]["y"] for b in range(B)], axis=0)
    return out.reshape(B, C, 64, 64).astype(np.float32)
